# revision 1
# baseline (speedup 1.0000x reference)
"""MiniMax-M2 MoE kernel for 8 Trainium2 NeuronCores.

Strategy (expert-parallel, sparse/routed):
  Phase A (device, token-parallel): router gate matmul + sigmoid + top-4
    selection + combine-weight renormalization. Each core routes T/8 tokens.
  Host (data movement only): gather tokens per expert from host-transposed
    xT, pad each expert slot to a static capacity, pre-transpose weights.
  Phase B (device, expert-parallel): per core, 2 experts' SwiGLU FFN over
    their gathered tokens, combine weight applied on device.
  Host: scatter-add per-expert outputs into the [T, H] result, in expert
    order (matches the reference scan accumulation order).
"""

import ml_dtypes
import numpy as np

import concourse.bass as bass
import concourse.tile as tile
from concourse import bacc, mybir
from concourse.bass_utils import run_bass_kernel_spmd

T, H, F, E, TOPK = 4096, 1024, 512, 16, 4
NCORES = 8
TLOC = T // NCORES  # tokens routed per core in phase A
F32 = mybir.dt.float32

_nc_cache: dict = {}
LAST_CAPS = (1408, 1024)  # caps used by the most recent kernel() call


def _build_phase_a(repeat: int = 1):
    """Router: per core, logits = x_slice @ gate_w.T; sigmoid; top-4 of
    (scores + bias); combine = renormalized raw scores at selected experts.

    Inputs per core:
      xt     [H, TLOC]  (host-transposed slice of hidden_states)
      gt     [H, E]     (host-transposed gate_w, replicated)
      bias128 [128, E]  (bias broadcast to 128 partitions, replicated)
    Output:
      comb   [TLOC, E]
    """
    nc = bacc.Bacc("TRN2", target_bir_lowering=False, debug=False,
                   num_devices=NCORES)
    xt = nc.dram_tensor("xt", [H, TLOC], F32, kind="ExternalInput").ap()
    gt = nc.dram_tensor("gt", [H, E], F32, kind="ExternalInput").ap()
    bias128 = nc.dram_tensor("bias128", [128, E], F32,
                             kind="ExternalInput").ap()
    comb_d = nc.dram_tensor("comb", [TLOC, E], F32, kind="ExternalOutput").ap()

    KC = H // 128  # contraction chunks
    NT = TLOC // 128  # token tiles per core

    with tile.TileContext(nc) as tc:
        with (
            tc.tile_pool(name="xt_p", bufs=1) as xt_p,
            tc.tile_pool(name="const_p", bufs=1) as const_p,
            tc.tile_pool(name="work_p", bufs=4) as work_p,
            tc.tile_pool(name="comb_p", bufs=2) as comb_p,
            tc.tile_pool(name="psum_p", bufs=4, space="PSUM") as psum_p,
        ):
            gt_sb = const_p.tile([128, KC, E], F32)
            nc.scalar.dma_start(
                gt_sb[:], gt.rearrange("(ko p) e -> p ko e", p=128))
            bias_sb = const_p.tile([128, E], F32)
            nc.scalar.dma_start(bias_sb[:], bias128[:])
            xt_r = xt.rearrange("(ko p) t -> p ko t", p=128)
            xt_sb = [xt_p.tile([128, TLOC], F32, tag=f"xt_{k}",
                               name=f"xt_sb_{k}")
                     for k in range(KC)]
            for k in range(KC):
                eng = nc.sync if k % 2 == 0 else nc.scalar
                eng.dma_start(xt_sb[k][:], xt_r[:, k])

            def body():
              comb_all = comb_p.tile([128, NT, E], F32)
              ps = psum_p.tile([128, NT, E], F32)
              for tt in range(NT):
                  for k in range(KC):
                      nc.tensor.matmul(
                          ps[:, tt, :],
                          lhsT=xt_sb[k][:, tt * 128:(tt + 1) * 128],
                          rhs=gt_sb[:, k],
                          start=(k == 0), stop=(k == KC - 1),
                      )
              # scores = sigmoid(logits), all NT token tiles at once
              sc = work_p.tile([128, NT, E], F32, tag="sc")
              nc.scalar.activation(
                  sc[:], ps[:], mybir.ActivationFunctionType.Sigmoid)
              biased = work_p.tile([128, NT, E], F32, tag="biased")
              nc.vector.tensor_tensor(
                  biased[:], sc[:],
                  bias_sb[:, None, :].to_broadcast([128, NT, E]),
                  mybir.AluOpType.add)
              # top-8 per row (descending); threshold = 4th largest
              m8 = work_p.tile([128, NT, 8], F32, tag="m8")
              sel = work_p.tile([128, NT, E], F32, tag="sel")
              for tt in range(NT):
                  nc.vector.max(m8[:, tt, :], biased[:, tt, :])
              for tt in range(NT):
                  nc.vector.tensor_scalar(
                      sel[:, tt, :], biased[:, tt, :],
                      m8[:, tt, TOPK - 1:TOPK], None,
                      op0=mybir.AluOpType.is_ge)
              picked = work_p.tile([128, NT, E], F32, tag="picked")
              nc.vector.tensor_mul(picked[:], sel[:], sc[:])
              denom = work_p.tile([128, NT], F32, tag="denom")
              nc.vector.reduce_sum(
                  denom[:], picked[:], axis=mybir.AxisListType.X)
              recip = work_p.tile([128, NT], F32, tag="recip")
              nc.vector.reciprocal(recip[:], denom[:])
              nc.vector.tensor_tensor(
                  comb_all[:], picked[:],
                  recip[:, :, None].to_broadcast([128, NT, E]),
                  mybir.AluOpType.mult)
              nc.sync.dma_start(
                  comb_d.rearrange("(n p) e -> p n e", p=128), comb_all[:])

            if repeat == 1:
                body()
            else:
                with tc.For_i(0, repeat, 1):
                    body()

    nc.compile()
    return nc


def _build_phase_b(caps: tuple[int, int], repeat: int = 1):
    """Expert FFN. Per core: 2 expert slots with static capacities caps.

    Inputs per core:
      w13t [2, H, 2F]  per-slot hstack(w1[e].T, w3[e].T)
      w2t  [2, F, H]   per-slot w2[e].T
      xgt  [H, CT]     gathered tokens (transposed), CT = caps[0]+caps[1]
      cvec [CT]        combine weight per gathered token (0 on padding)
    Output:
      yg   [CT, H]     combine-weighted expert outputs per gathered token
    """
    CT = sum(caps)
    assert CT % 128 == 0
    BF16 = mybir.dt.bfloat16
    nc = bacc.Bacc("TRN2", target_bir_lowering=False, debug=False,
                   num_devices=NCORES)
    w13 = nc.dram_tensor("w13t", [2, H, 2 * F], BF16,
                         kind="ExternalInput").ap()
    w2t = nc.dram_tensor("w2t", [2, F, H], BF16, kind="ExternalInput").ap()
    xgt = nc.dram_tensor("xgt", [H, CT], BF16, kind="ExternalInput").ap()
    cvec = nc.dram_tensor("cvec", [128, CT // 128], F32,
                          kind="ExternalInput").ap()
    yg = nc.dram_tensor("yg", [CT, H], BF16, kind="ExternalOutput").ap()

    KC = H // 128   # stage-1 contraction chunks
    FC = F // 128   # stage-2 contraction chunks (= hT partition chunks)
    xgt_r = xgt.rearrange("(ko p) t -> p ko t", p=128)

    def chunk_sizes(cap):
        # split cap into multiples of 128, each <=512, reasonably even
        sizes = []
        rem = cap
        while rem > 0:
            if rem > 512 and rem % 512 == 128:
                s = 384  # avoid leaving a 128 tail
            else:
                s = min(512, rem)
            sizes.append(s)
            rem -= s
        return sizes

    with tile.TileContext(nc) as tc:
        with (
            tc.tile_pool(name="w13_p", bufs=2) as w13_p,
            tc.tile_pool(name="w2_p", bufs=2) as w2_p,
            tc.tile_pool(name="xg_p", bufs=2) as xg_p,
            tc.tile_pool(name="ht_p", bufs=2) as ht_p,
            tc.tile_pool(name="sg_p", bufs=2) as sg_p,
            tc.tile_pool(name="y_p", bufs=3) as y_p,
            tc.tile_pool(name="c_p", bufs=1) as c_p,
            tc.tile_pool(name="ps", bufs=8, space="PSUM") as ps_pool,
        ):
            c_sb = c_p.tile([128, CT // 128], F32)
            nc.scalar.dma_start(c_sb[:], cvec[:])

            def body():
              for s in range(2):
                  cap = caps[s]
                  off = sum(caps[:s])
                  chunks = chunk_sizes(cap)

                  def load_xg(t0, tl):
                      tiles = [xg_p.tile([128, 512], BF16, tag=f"xg_{k}",
                                         name=f"xg_sb_{t0}_{k}")
                               for k in range(KC)]
                      for k in range(KC):
                          nc.sync.dma_start(
                              tiles[k][:, :tl], xgt_r[:, k, t0:t0 + tl])
                      return tiles

                # per-k-chunk weight tiles, interleaved with the first token
                # chunk's loads, so matmuls start after ~2 small DMAs instead
                # of after the full weight matrix
                  w13_sb = [w13_p.tile([128, 2 * F], BF16, tag=f"w13_{k}",
                                       name=f"w13_sb_{s}_{k}")
                            for k in range(KC)]
                  xg_first = [xg_p.tile([128, 512], BF16, tag=f"xg_{k}",
                                        name=f"xg_sb_first{s}_{k}")
                              for k in range(KC)]
                  for k in range(KC):
                      nc.sync.dma_start(
                          w13_sb[k][:],
                          w13[s, k * 128:(k + 1) * 128, :])
                      nc.scalar.dma_start(
                          xg_first[k][:, :chunks[0]],
                          xgt_r[:, k, off:off + chunks[0]])
                  w2_sb = w2_p.tile([128, FC, H], BF16, tag="w2")
                  nc.scalar.dma_start(
                      w2_sb[:], w2t[s].rearrange("(ko p) h -> p ko h", p=128))

                  tch0 = 0
                  for ci, tl in enumerate(chunks):
                      t0 = off + tch0
                      tch0 += tl
                      xg_sb = xg_first if ci == 0 else load_xg(t0, tl)
                      ht_sb = ht_p.tile([128, FC, 512], BF16, tag="ht")
                    # stage 1: hT[f, t] = silu(xg@w1.T).T * (xg@w3.T).T
                    # For the ramp chunk (first of slot 0) run k OUTER so the
                    # PE consumes weight/activation chunks as they stream in;
                    # otherwise k inner (denser PSUM reuse).
                      ps_gs = [ps_pool.tile([128, 512], F32, tag="ps",
                                            name=f"ps_g_{s}_{t0}_{fi}")
                               for fi in range(FC)]
                      ps_us = [ps_pool.tile([128, 512], F32, tag="ps",
                                            name=f"ps_u_{s}_{t0}_{fi}")
                               for fi in range(FC)]

                      def mm_s1(fi, k, ps_g, ps_u):
                          nc.tensor.matmul(
                              ps_g[:, :tl],
                              lhsT=w13_sb[k][:, fi * 128:(fi + 1) * 128],
                              rhs=xg_sb[k][:, :tl],
                              start=(k == 0), stop=(k == KC - 1))
                          nc.tensor.matmul(
                              ps_u[:, :tl],
                              lhsT=w13_sb[k][:,
                                            F + fi * 128:F + (fi + 1) * 128],
                              rhs=xg_sb[k][:, :tl],
                              start=(k == 0), stop=(k == KC - 1))

                      if s == 0 and ci == 0:
                          for k in range(KC):
                              for fi in range(FC):
                                  mm_s1(fi, k, ps_gs[fi], ps_us[fi])
                      else:
                          for fi in range(FC):
                              for k in range(KC):
                                  mm_s1(fi, k, ps_gs[fi], ps_us[fi])
                      for fi in range(FC):
                          sg = sg_p.tile([128, 512], F32, tag="sg")
                          nc.scalar.activation(
                              sg[:, :tl], ps_gs[fi][:, :tl],
                              mybir.ActivationFunctionType.Silu)
                          nc.vector.tensor_mul(
                              ht_sb[:, fi, :tl], sg[:, :tl], ps_us[fi][:, :tl])
                    # stage 2: y[t, h] = c[t] * sum_f hT[f, t] * w2T[f, h]
                      for tt0 in range(0, tl, 128):
                          ttl = min(128, tl - tt0)
                          cidx = (t0 + tt0) // 128
                          y_sb = y_p.tile([128, H], BF16, tag="y")
                          for hh in range(2):
                              ps_y = ps_pool.tile([128, 512], F32, tag="ps")
                              for kf in range(FC):
                                  nc.tensor.matmul(
                                      ps_y[:ttl],
                                      lhsT=ht_sb[:, kf, tt0:tt0 + ttl],
                                      rhs=w2_sb[:, kf, hh * 512:(hh + 1) * 512],
                                      start=(kf == 0), stop=(kf == FC - 1))
                              nc.vector.tensor_scalar(
                                  y_sb[:ttl, hh * 512:(hh + 1) * 512],
                                  ps_y[:ttl], c_sb[:, cidx:cidx + 1], None,
                                  op0=mybir.AluOpType.mult)
                          nc.scalar.dma_start(
                              yg[t0 + tt0:t0 + tt0 + ttl, :], y_sb[:ttl, :])

            if repeat == 1:
                body()
            else:
                with tc.For_i(0, repeat, 1):
                    body()

    nc.compile()
    return nc


def _phase_a_nc():
    key = ("a",)
    if key not in _nc_cache:
        _nc_cache[key] = _build_phase_a()
    return _nc_cache[key]


def _phase_b_nc(caps):
    key = ("b", caps)
    if key not in _nc_cache:
        _nc_cache[key] = _build_phase_b(caps)
    return _nc_cache[key]


def _pad128(n: int) -> int:
    return max(128, (n + 127) // 128 * 128)


def kernel(hidden_states, gate_w, bias, w1, w3, w2):
    x = np.ascontiguousarray(np.asarray(hidden_states, dtype=np.float32))
    gate_w = np.asarray(gate_w, dtype=np.float32)
    bias = np.asarray(bias, dtype=np.float32)
    w1 = np.asarray(w1, dtype=np.float32)
    w3 = np.asarray(w3, dtype=np.float32)
    w2 = np.asarray(w2, dtype=np.float32)

    xT = np.ascontiguousarray(x.T)                      # [H, T]
    gT = np.ascontiguousarray(gate_w.T)                 # [H, E]
    bias128 = np.ascontiguousarray(
        np.broadcast_to(bias[None, :], (128, E)))

    # ---- Phase A: routing on device (token-parallel) ----
    ncA = _phase_a_nc()
    in_maps_a = [
        {
            "xt": np.ascontiguousarray(xT[:, c * TLOC:(c + 1) * TLOC]),
            "gt": gT,
            "bias128": bias128,
        }
        for c in range(NCORES)
    ]
    resA = run_bass_kernel_spmd(ncA, in_maps_a, core_ids=list(range(NCORES)))
    combine = np.concatenate(
        [resA.results[c]["comb"] for c in range(NCORES)], axis=0)  # [T, E]

    # ---- Host dispatch: order experts by load, two slots per core ----
    idx_per_e = [np.nonzero(combine[:, e] > 0.0)[0] for e in range(E)]
    counts = np.array([len(ix) for ix in idx_per_e])
    order = np.argsort(-counts, kind="stable")          # experts by load desc
    slot0 = [int(order[c]) for c in range(NCORES)]      # heavy experts
    slot1 = [int(order[NCORES + c]) for c in range(NCORES)]  # light experts
    C0 = _pad128(int(counts[order[:NCORES]].max()))
    C1 = _pad128(int(counts[order[NCORES:]].max()))
    caps = (C0, C1)
    global LAST_CAPS
    LAST_CAPS = caps
    CT = C0 + C1
    xT16 = xT.astype(ml_dtypes.bfloat16)

    in_maps_b = []
    for c in range(NCORES):
        pair = (slot0[c], slot1[c])
        idx_pad = np.zeros(CT, dtype=np.int64)
        cv = np.zeros(CT, dtype=np.float32)
        for s, e in enumerate(pair):
            off = s * C0
            ix = idx_per_e[e]
            idx_pad[off:off + len(ix)] = ix
            cv[off:off + len(ix)] = combine[ix, e]
        xgt = np.ascontiguousarray(xT16[:, idx_pad])    # [H, CT] bf16
        w13t = np.stack([
            np.ascontiguousarray(
                np.concatenate([w1[e].T, w3[e].T], axis=1))
            for e in pair]).astype(ml_dtypes.bfloat16)   # [2, H, 2F]
        w2t = np.stack(
            [np.ascontiguousarray(w2[e].T) for e in pair]
        ).astype(ml_dtypes.bfloat16)
        cv_tiled = np.ascontiguousarray(cv.reshape(CT // 128, 128).T)
        in_maps_b.append(
            {"w13t": w13t, "w2t": w2t, "xgt": xgt, "cvec": cv_tiled})

    # ---- Phase B: expert FFN on device (expert-parallel) ----
    ncB = _phase_b_nc(caps)
    resB = run_bass_kernel_spmd(ncB, in_maps_b, core_ids=list(range(NCORES)))

    # ---- Host combine: scatter-add in expert order ----
    out = np.zeros((T, H), dtype=np.float32)
    where = {}
    for c in range(NCORES):
        where[slot0[c]] = (c, 0)
        where[slot1[c]] = (c, C0)
    for e in range(E):
        c, off = where[e]
        ix = idx_per_e[e]
        if len(ix):
            out[ix] += resB.results[c]["yg"][off:off + len(ix)
                                             ].astype(np.float32)
    return out



# revision 12
# speedup vs baseline: 1.1360x; 1.1360x over previous
"""MiniMax-M2 MoE kernel for 8 Trainium2 NeuronCores.

Single-launch expert-parallel design:
  Host (data movement / dispatch only): fp32 routing decides WHICH tokens go
    to WHICH expert (indices only), gathers tokens per expert into two static
    slots per core (2 experts per core, capacities = exact max expert load),
    pre-transposes/casts weights to bf16.
  Device (all output-value arithmetic, one SPMD launch):
    - per slot, recompute router scores for the slot's gathered tokens
      (logits -> sigmoid -> top-4 threshold on bias-corrected scores ->
      renormalized combine weight of the slot's own expert; the host permutes
      the gate matrix per core so the slot expert is always column 0/1),
    - SwiGLU FFN (bf16 matmuls) and combine-weight scaling.  silu(g) is
      computed as g * sigmoid(g) so the Activation engine only ever needs the
      sigmoid table (one LoadActFuncSet instead of thrashing Silu<->Sigmoid).
  Host: scatter-add per-expert outputs into [T, H] in expert order.
"""

import math

import ml_dtypes
import numpy as np

import concourse.bass as bass
import concourse.tile as tile
from concourse import bacc, mybir
from concourse.bass_utils import run_bass_kernel_spmd

T, H, F, E, TOPK = 4096, 1024, 512, 16, 4
NCORES = 8
KC = H // 128   # contraction chunks (hidden dim)
FC = F // 128   # stage-2 contraction chunks
F32 = mybir.dt.float32
BF16 = mybir.dt.bfloat16

_nc_cache: dict = {}
LAST_CAPS = (1321, 1004)  # caps used by the most recent kernel() call


def _chunk_sizes(cap: int, rem_first: bool) -> list[int]:
    """Split cap into <=512-sized chunks; remainder first or last."""
    n_full, rem = divmod(cap, 512)
    sizes = [512] * n_full
    if rem:
        if rem_first:
            sizes = [rem] + sizes
        else:
            sizes = sizes + [rem]
    return sizes


def _build_moe(caps: tuple[int, int]):
    """One-launch MoE FFN + on-device combine weights.

    Inputs per core:
      w13t  [2, H, 2F] bf16  per-slot hstack(w1[e].T, w3[e].T)
      w2t   [2, F, H]  bf16  per-slot w2[e].T
      xgt   [H, CT]    bf16  gathered tokens (transposed), CT = caps[0]+caps[1]
      gtp   [H, E]     bf16  gate_w.T, columns permuted so that column s is
                             slot s's expert
      biasp [128, E]   f32   e_score_correction_bias, same permutation,
                             broadcast to 128 partitions
    Output:
      yg    [CT, H]    bf16  combine-weighted expert outputs per gathered token
    """
    CT = sum(caps)
    chunk_lists = [_chunk_sizes(caps[0], rem_first=False),
                   _chunk_sizes(caps[1], rem_first=False)]
    ntiles_total = sum(math.ceil(tl / 128)
                       for chunks in chunk_lists for tl in chunks)

    nc = bacc.Bacc("TRN2", target_bir_lowering=False, debug=False,
                   num_devices=NCORES)
    w13 = nc.dram_tensor("w13t", [2, H, 2 * F], BF16,
                         kind="ExternalInput").ap()
    w2t = nc.dram_tensor("w2t", [2, F, H], BF16, kind="ExternalInput").ap()
    xgt = nc.dram_tensor("xgt", [H, CT], BF16, kind="ExternalInput").ap()
    gtp = nc.dram_tensor("gtp", [H, E], BF16, kind="ExternalInput").ap()
    biasp = nc.dram_tensor("biasp", [128, E], F32, kind="ExternalInput").ap()
    yg = nc.dram_tensor("yg", [CT, H], BF16, kind="ExternalOutput").ap()

    xgt_r = xgt.rearrange("(ko p) t -> p ko t", p=128)
    SIG = mybir.ActivationFunctionType.Sigmoid

    with tile.TileContext(nc) as tc:
        with (
            tc.tile_pool(name="const_p", bufs=1) as const_p,
            tc.tile_pool(name="w13_p", bufs=2) as w13_p,
            tc.tile_pool(name="w2_p", bufs=2) as w2_p,
            tc.tile_pool(name="xg_p", bufs=3) as xg_p,
            tc.tile_pool(name="ht_p", bufs=2) as ht_p,
            tc.tile_pool(name="sg_p", bufs=2) as sg_p,
            tc.tile_pool(name="y_p", bufs=3) as y_p,
            tc.tile_pool(name="work_p", bufs=2) as work_p,
            tc.tile_pool(name="ps", bufs=4, space="PSUM") as ps_pool,
        ):
            gt_sb = const_p.tile([128, KC, E], BF16)
            bias_sb = const_p.tile([128, E], F32)
            w_sb = const_p.tile([128, ntiles_total], F32)

            def routing(xg_sb, tl, nt, s, ci, jglob):
                """Combine weight of this slot's expert for one token chunk."""
                ps_r = ps_pool.tile([128, nt, E], F32, tag="psr",
                                    bufs=2, name=f"ps_r_{s}_{ci}")
                # partial last tile leaves rows >= ttl unwritten by the
                # matmuls; zero-fill so batched reads are fully defined
                nc.vector.memset(ps_r[:, :nt, :], 0.0)
                for j in range(nt):
                    tt0 = j * 128
                    ttl = min(128, tl - tt0)
                    for k in range(KC):
                        nc.tensor.matmul(
                            ps_r[:ttl, j, :],
                            lhsT=xg_sb[k][:, tt0:tt0 + ttl],
                            rhs=gt_sb[:, k, :],
                            start=(k == 0), stop=(k == KC - 1))
                sc = work_p.tile([128, nt, E], F32, tag="sc",
                                 name=f"sc_{s}_{ci}", padded_shape=[128, 4, E])
                nc.scalar.activation(sc[:, :nt, :], ps_r[:, :nt, :], SIG)
                biased = work_p.tile([128, nt, E], F32, tag="biased",
                                     name=f"biased_{s}_{ci}",
                                     padded_shape=[128, 4, E])
                nc.vector.tensor_tensor(
                    biased[:, :nt, :], sc[:, :nt, :],
                    bias_sb[:, None, :].to_broadcast([128, nt, E]),
                    mybir.AluOpType.add)
                m8 = work_p.tile([128, nt, 8], F32, tag="m8",
                                 name=f"m8_{s}_{ci}", padded_shape=[128, 4, 8])
                sel = work_p.tile([128, nt, E], F32, tag="sel",
                                  name=f"sel_{s}_{ci}",
                                  padded_shape=[128, 4, E])
                for j in range(nt):
                    nc.vector.max(m8[:, j, :], biased[:, j, :])
                for j in range(nt):
                    nc.vector.tensor_scalar(
                        sel[:, j, :], biased[:, j, :],
                        m8[:, j, TOPK - 1:TOPK], None,
                        op0=mybir.AluOpType.is_ge)
                picked = work_p.tile([128, nt, E], F32, tag="picked",
                                     name=f"picked_{s}_{ci}",
                                     padded_shape=[128, 4, E])
                nc.vector.tensor_mul(
                    picked[:, :nt, :], sel[:, :nt, :], sc[:, :nt, :])
                denom = work_p.tile([128, nt], F32, tag="denom",
                                    name=f"denom_{s}_{ci}",
                                    padded_shape=[128, 4])
                nc.vector.reduce_sum(
                    denom[:, :nt], picked[:, :nt, :], axis=mybir.AxisListType.X)
                recip = work_p.tile([128, nt], F32, tag="recip",
                                    name=f"recip_{s}_{ci}",
                                    padded_shape=[128, 4])
                nc.vector.reciprocal(recip[:, :nt], denom[:, :nt])
                # slot expert score is column s (host permutation)
                nc.vector.tensor_mul(
                    w_sb[:, jglob:jglob + nt], sc[:, :nt, s], recip[:, :nt])

            def evac_stage1(ps_g, ps_u, ht_sb, fi, tl):
                """ht[:, fi, :tl] = silu(g) * u = g * sigmoid(g) * u."""
                sgm = sg_p.tile([128, 512], F32, tag="sgm", name=f"sgm_{fi}")
                nc.scalar.activation(sgm[:, :tl], ps_g[:, :tl], SIG)
                gsg = sg_p.tile([128, 512], F32, tag="gsg", name=f"gsg_{fi}")
                nc.vector.tensor_mul(gsg[:, :tl], sgm[:, :tl], ps_g[:, :tl])
                nc.vector.tensor_mul(
                    ht_sb[:, fi, :tl], gsg[:, :tl], ps_u[:, :tl])

            jglob = 0
            for s in range(2):
                cap = caps[s]
                off = sum(caps[:s])
                chunks = chunk_lists[s]

                # k=0 weights split into g/u halves so the first matmul's DMA
                # dependency is small; k>=1 combined to halve the issue count
                w13g0 = w13_p.tile([128, F], BF16, tag="w13g0",
                                   name=f"w13g0_{s}")
                w13u0 = w13_p.tile([128, F], BF16, tag="w13u0",
                                   name=f"w13u0_{s}")
                nc.sync.dma_start(w13g0[:], w13[s, 0:128, 0:F])
                nc.sync.dma_start(w13u0[:], w13[s, 0:128, F:2 * F])
                w13k = [None] + [w13_p.tile([128, 2 * F], BF16,
                                            tag=f"w13_{k}",
                                            name=f"w13_sb_{s}_{k}")
                                 for k in range(1, KC)]
                for k in range(1, KC):
                    nc.sync.dma_start(
                        w13k[k][:], w13[s, k * 128:(k + 1) * 128, :])

                def gv(k, fi):
                    if k == 0:
                        return w13g0[:, fi * 128:(fi + 1) * 128]
                    return w13k[k][:, fi * 128:(fi + 1) * 128]

                def uv(k, fi):
                    if k == 0:
                        return w13u0[:, fi * 128:(fi + 1) * 128]
                    return w13k[k][:, F + fi * 128:F + (fi + 1) * 128]

                tch0 = 0
                for ci, tl in enumerate(chunks):
                    t0 = off + tch0
                    tch0 += tl
                    nt = math.ceil(tl / 128)
                    ramp = (s == 0 and ci == 0)

                    xg_big = xg_p.tile([128, KC, 512], BF16, tag="xg",
                                       name=f"xg_sb_{s}_{ci}")
                    xg_sb = [xg_big[:, k, :] for k in range(KC)]
                    if ramp:
                        # per-k DMAs so the PE can consume k-chunks as they
                        # stream in during the cold start
                        for k in range(KC):
                            nc.scalar.dma_start(
                                xg_big[:, k, :tl], xgt_r[:, k, t0:t0 + tl])
                    else:
                        nc.sync.dma_start(
                            xg_big[:, :, :tl], xgt_r[:, :, t0:t0 + tl])
                    if ramp:
                        # routing consts + slot-0 w2 AFTER the ramp-critical
                        # xg tiles (a big early w2 transfer would stall the
                        # first matmuls behind it on the shared DMA engines)
                        nc.scalar.dma_start(
                            gt_sb[:],
                            gtp.rearrange("(ko p) e -> p ko e", p=128))
                        nc.scalar.dma_start(bias_sb[:], biasp[:])
                    if ci == 0:
                        w2_sb = w2_p.tile([128, FC, H], BF16, tag="w2",
                                          name=f"w2_sb_{s}")
                        nc.scalar.dma_start(
                            w2_sb[:],
                            w2t[s].rearrange("(ko p) h -> p ko h", p=128))

                    ht_sb = ht_p.tile([128, FC, 512], BF16, tag="ht")

                    if not ramp:
                        # routing first: its ACT+DVE chain then completes
                        # during stage 1, well before stage 2 consumes w_sb
                        routing(xg_sb, tl, nt, s, ci, jglob)

                    # ---- stage 1: hT[f,t] = silu(x@w1.T).T * (x@w3.T).T ----
                    if ramp:
                        # k OUTER across all fi: the PE consumes each
                        # weight/activation k-chunk as it streams in.
                        # 8 live PSUM tiles across the three tags.
                        tags = ["ps1", "ps1", "ps1", "psy",
                                "psy", "psy", "psr", "psr"]
                        ps8 = [ps_pool.tile([128, 512], F32, tag=tags[i],
                                            bufs=(2 if tags[i] == "psr"
                                                  else 3),
                                            name=f"ps_ramp_{i}")
                               for i in range(8)]
                        ps_gs = ps8[0::2]
                        ps_us = ps8[1::2]
                        for k in range(KC):
                            for fi in range(FC):
                                nc.tensor.matmul(
                                    ps_gs[fi][:, :tl], lhsT=gv(k, fi),
                                    rhs=xg_sb[k][:, :tl],
                                    start=(k == 0), stop=(k == KC - 1))
                                nc.tensor.matmul(
                                    ps_us[fi][:, :tl], lhsT=uv(k, fi),
                                    rhs=xg_sb[k][:, :tl],
                                    start=(k == 0), stop=(k == KC - 1))
                        for fi in range(FC):
                            evac_stage1(ps_gs[fi], ps_us[fi], ht_sb, fi, tl)
                        # ramp routing last (needs every xg k-chunk anyway)
                        routing(xg_sb, tl, nt, s, ci, jglob)
                    else:
                        # fi sequential, k inner: only 2 PSUM tiles live
                        for fi in range(FC):
                            ps_g = ps_pool.tile([128, 512], F32, tag="ps1",
                                                bufs=3,
                                                name=f"ps_g_{s}_{ci}_{fi}")
                            ps_u = ps_pool.tile([128, 512], F32, tag="ps1",
                                                bufs=3,
                                                name=f"ps_u_{s}_{ci}_{fi}")
                            for k in range(KC):
                                nc.tensor.matmul(
                                    ps_g[:, :tl], lhsT=gv(k, fi),
                                    rhs=xg_sb[k][:, :tl],
                                    start=(k == 0), stop=(k == KC - 1))
                                nc.tensor.matmul(
                                    ps_u[:, :tl], lhsT=uv(k, fi),
                                    rhs=xg_sb[k][:, :tl],
                                    start=(k == 0), stop=(k == KC - 1))
                            evac_stage1(ps_g, ps_u, ht_sb, fi, tl)

                    # ---- stage 2: y[t,h] = w[t] * sum_f hT[f,t]*w2T[f,h] ---
                    for j in range(nt):
                        tt0 = j * 128
                        ttl = min(128, tl - tt0)
                        y_sb = y_p.tile([128, H], BF16, tag="y")
                        for hh in range(2):
                            ps_y = ps_pool.tile([128, 512], F32, tag="psy",
                                                bufs=3,
                                                name=f"ps_y_{s}_{ci}_{j}_{hh}")
                            for kf in range(FC):
                                nc.tensor.matmul(
                                    ps_y[:ttl],
                                    lhsT=ht_sb[:, kf, tt0:tt0 + ttl],
                                    rhs=w2_sb[:, kf, hh * 512:(hh + 1) * 512],
                                    start=(kf == 0), stop=(kf == FC - 1))
                            nc.vector.tensor_scalar(
                                y_sb[:ttl, hh * 512:(hh + 1) * 512],
                                ps_y[:ttl],
                                w_sb[:ttl, jglob + j:jglob + j + 1], None,
                                op0=mybir.AluOpType.mult)
                        nc.sync.dma_start(
                            yg[t0 + tt0:t0 + tt0 + ttl, :], y_sb[:ttl, :])
                    jglob += nt

    nc.compile()
    return nc


def _moe_nc(caps):
    key = ("moe", caps)
    if key not in _nc_cache:
        _nc_cache[key] = _build_moe(caps)
    return _nc_cache[key]


def kernel(hidden_states, gate_w, bias, w1, w3, w2):
    x = np.ascontiguousarray(np.asarray(hidden_states, dtype=np.float32))
    gate_w = np.asarray(gate_w, dtype=np.float32)
    bias = np.asarray(bias, dtype=np.float32)
    w1 = np.asarray(w1, dtype=np.float32)
    w3 = np.asarray(w3, dtype=np.float32)
    w2 = np.asarray(w2, dtype=np.float32)

    # ---- Host dispatch: fp32 routing decides token->expert placement ----
    logits = x @ gate_w.T                                # [T, E]
    scores = 1.0 / (1.0 + np.exp(-logits))
    biased = scores + bias[None, :]
    topi = np.argpartition(-biased, TOPK - 1, axis=1)[:, :TOPK]  # [T, K] sets
    sel = np.zeros((T, E), dtype=bool)
    sel[np.arange(T)[:, None], topi] = True
    idx_per_e = [np.nonzero(sel[:, e])[0] for e in range(E)]
    counts = np.array([len(ix) for ix in idx_per_e])
    order = np.argsort(-counts, kind="stable")           # experts by load desc
    slot0 = [int(order[c]) for c in range(NCORES)]
    slot1 = [int(order[NCORES + c]) for c in range(NCORES)]
    C0 = int(counts[order[:NCORES]].max())
    C1 = int(counts[order[NCORES:]].max())
    caps = (C0, C1)
    global LAST_CAPS
    LAST_CAPS = caps
    CT = C0 + C1

    xT = np.ascontiguousarray(x.T)                       # [H, T]
    xT16 = xT.astype(ml_dtypes.bfloat16)
    gT16 = np.ascontiguousarray(gate_w.T).astype(ml_dtypes.bfloat16)

    in_maps = []
    for c in range(NCORES):
        pair = (slot0[c], slot1[c])
        idx_pad = np.zeros(CT, dtype=np.int64)
        for s, e in enumerate(pair):
            off = s * C0
            ix = idx_per_e[e]
            idx_pad[off:off + len(ix)] = ix
        xgt = np.ascontiguousarray(xT16[:, idx_pad])     # [H, CT] bf16
        w13t = np.stack([
            np.ascontiguousarray(
                np.concatenate([w1[e].T, w3[e].T], axis=1))
            for e in pair]).astype(ml_dtypes.bfloat16)   # [2, H, 2F]
        w2t = np.stack(
            [np.ascontiguousarray(w2[e].T) for e in pair]
        ).astype(ml_dtypes.bfloat16)
        perm = list(pair) + [e for e in range(E) if e not in pair]
        gtp = np.ascontiguousarray(gT16[:, perm])        # [H, E] bf16
        biasp = np.ascontiguousarray(
            np.broadcast_to(bias[perm][None, :], (128, E))).astype(np.float32)
        in_maps.append(
            {"w13t": w13t, "w2t": w2t, "xgt": xgt, "gtp": gtp,
             "biasp": biasp})

    # ---- Single SPMD launch: routing weights + expert FFN ----
    ncB = _moe_nc(caps)
    res = run_bass_kernel_spmd(ncB, in_maps, core_ids=list(range(NCORES)))

    # ---- Host combine: scatter-add in expert order ----
    out = np.zeros((T, H), dtype=np.float32)
    where = {}
    for c in range(NCORES):
        where[slot0[c]] = (c, 0)
        where[slot1[c]] = (c, C0)
    for e in range(E):
        c, off = where[e]
        ix = idx_per_e[e]
        if len(ix):
            out[ix] += res.results[c]["yg"][off:off + len(ix)
                                            ].astype(np.float32)
    return out


# revision 18
# speedup vs baseline: 1.2021x; 1.0582x over previous
"""MiniMax-M2 MoE kernel for 8 Trainium2 NeuronCores.

Single-launch expert-parallel design:
  Host (data movement / dispatch only): fp32 routing decides WHICH tokens go
    to WHICH expert (indices only), gathers tokens per expert into two static
    slots per core (2 experts per core, capacities = exact max expert load),
    pre-transposes/casts weights to bf16.
  Device (all output-value arithmetic, one SPMD launch):
    - per slot, recompute router scores for the slot's gathered tokens
      (logits -> sigmoid -> top-4 threshold on bias-corrected scores ->
      renormalized combine weight of the slot's own expert; the host permutes
      the gate matrix per core so the slot expert is always column 0/1),
    - SwiGLU FFN (bf16 matmuls) and combine-weight scaling.  silu(g) is
      computed as g * sigmoid(g) so the Activation engine only ever needs the
      sigmoid table (one LoadActFuncSet instead of thrashing Silu<->Sigmoid).
  Host: scatter-add per-expert outputs into [T, H] in expert order.
"""

import math

import ml_dtypes
import numpy as np

import concourse.bass as bass
import concourse.tile as tile
from concourse import bacc, mybir
from concourse.bass_utils import run_bass_kernel_spmd

T, H, F, E, TOPK = 4096, 1024, 512, 16, 4
NCORES = 8
KC = H // 128   # contraction chunks (hidden dim)
FC = F // 128   # stage-2 contraction chunks
F32 = mybir.dt.float32
BF16 = mybir.dt.bfloat16

_nc_cache: dict = {}
LAST_CAPS = (832, 492, 512, 354)  # caps used by the most recent kernel() call


def _plan_slots(counts: np.ndarray):
    """Choose per-core slot capacities and expert-piece placement.

    Experts are cut into at most two pieces (primary, remainder).  Slot type
    0 holds heavy-expert primaries (cap A), type 2 light primaries (cap C);
    the remainders are ranked and split between types 1 and 3.  The (A, C)
    cut points are searched to minimize modeled PE time: stage-1 cost scales
    with total capacity, stage-2/routing with ceil(cap/128) tiles.

    Returns (caps, placement) where placement[core] is a list of
    (expert, tok_start, length) per slot (length may be 0).
    """
    E_ = len(counts)
    order = np.argsort(-counts, kind="stable")
    heavy = [int(e) for e in order[:NCORES]]
    light = [int(e) for e in order[NCORES:]]
    c0 = int(counts[heavy[0]])
    c8 = int(counts[light[0]])

    def plan_cost(caps):
        ct = sum(caps)
        tiles = sum(math.ceil(cp / 128) for cp in caps if cp)
        return 64 * ct + (8 * 512 + 8 * E_) * tiles

    def build(A, C):
        pieces_b = []  # (expert, start, len) remainders
        for e in heavy:
            if counts[e] > A:
                pieces_b.append((e, A, int(counts[e]) - A))
        for e in light:
            if counts[e] > C:
                pieces_b.append((e, C, int(counts[e]) - C))
        if len(pieces_b) > 2 * NCORES:
            return None
        pieces_b.sort(key=lambda p: -p[2])
        bs = pieces_b[:NCORES]
        ds = pieces_b[NCORES:]
        a = min(c0, A)
        b = bs[0][2] if bs else 0
        c = min(c8, C)
        d = ds[0][2] if ds else 0
        caps = (a, b, c, d)
        # piece -> core assignment avoiding same expert twice on one core
        placement = [[None] * 4 for _ in range(NCORES)]
        for i in range(NCORES):
            placement[i][0] = (heavy[i], 0, min(int(counts[heavy[i]]), A))
            placement[i][2] = (light[i], 0, min(int(counts[light[i]]), C))
        for sl, plist in ((1, bs), (3, ds)):
            free = set(range(NCORES))
            for e, st, ln in plist:
                cand = [i for i in free
                        if e != placement[i][0][0] and e != placement[i][2][0]
                        and (placement[i][1] is None or
                             placement[i][1][0] != e)]
                if not cand:
                    return None
                i = cand[0]
                free.discard(i)
                placement[i][sl] = (e, st, ln)
        return caps, placement

    best = None
    lo_a = (c0 + 1) // 2
    lo_c = (c8 + 1) // 2
    cands = [(c0, c8)]
    for A in range(lo_a, c0 + 1, 2):
        for C in range(lo_c, c8 + 1, 2):
            cands.append((A, C))
    for A, C in cands:
        got = build(A, C)
        if got is None:
            continue
        caps, placement = got
        cost = plan_cost(caps)
        if best is None or cost < best[0]:
            best = (cost, caps, placement)
    _, caps, placement = best
    # drop zero-cap slots; fill empty kept slots with a zero-length piece of
    # some expert not already used by that core (perm needs distinct experts)
    keep = [si for si in range(4) if caps[si] > 0]
    caps_k = tuple(caps[si] for si in keep)
    placement_k = []
    for pl in placement:
        row = []
        used = {p[0] for p in pl if p is not None}
        for si in keep:
            p = pl[si]
            if p is None:
                e_fill = next(e for e in range(E_) if e not in used)
                used.add(e_fill)
                p = (e_fill, 0, 0)
            row.append(p)
        placement_k.append(row)
    return caps_k, placement_k


def _chunk_sizes(cap: int, rem_first: bool) -> list[int]:
    """Split cap into <=512-sized chunks; remainder first or last."""
    n_full, rem = divmod(cap, 512)
    sizes = [512] * n_full
    if rem:
        if rem_first:
            sizes = [rem] + sizes
        else:
            sizes = sizes + [rem]
    return sizes


def _build_moe(caps: tuple[int, ...]):
    """One-launch MoE FFN + on-device combine weights.

    Inputs per core (S = len(caps) expert slots):
      w13t  [S, H, 2F] bf16  per-slot hstack(w1[e].T, w3[e].T)
      w2t   [S, F, H]  bf16  per-slot w2[e].T
      xgt   [H, CT]    bf16  gathered tokens (transposed), CT = sum(caps)
      gtp   [H, E]     bf16  gate_w.T, columns permuted so that column s is
                             slot s's expert
      biasp [128, E]   f32   e_score_correction_bias, same permutation,
                             broadcast to 128 partitions
    Output:
      yg    [CT, H]    bf16  combine-weighted expert outputs per gathered token
    """
    S = len(caps)
    CT = sum(caps)
    chunk_lists = [_chunk_sizes(cap, rem_first=False) for cap in caps]
    ntiles_total = sum(math.ceil(tl / 128)
                       for chunks in chunk_lists for tl in chunks)

    nc = bacc.Bacc("TRN2", target_bir_lowering=False, debug=False,
                   num_devices=NCORES)
    w13 = nc.dram_tensor("w13t", [S, H, 2 * F], BF16,
                         kind="ExternalInput").ap()
    w2t = nc.dram_tensor("w2t", [S, F, H], BF16, kind="ExternalInput").ap()
    xgt = nc.dram_tensor("xgt", [H, CT], BF16, kind="ExternalInput").ap()
    gtp = nc.dram_tensor("gtp", [H, E], BF16, kind="ExternalInput").ap()
    biasp = nc.dram_tensor("biasp", [128, E], F32, kind="ExternalInput").ap()
    yg = nc.dram_tensor("yg", [CT, H], BF16, kind="ExternalOutput").ap()

    xgt_r = xgt.rearrange("(ko p) t -> p ko t", p=128)
    SIG = mybir.ActivationFunctionType.Sigmoid

    with tile.TileContext(nc) as tc:
        with (
            tc.tile_pool(name="const_p", bufs=1) as const_p,
            tc.tile_pool(name="w13_p", bufs=2) as w13_p,
            tc.tile_pool(name="w2_p", bufs=2) as w2_p,
            tc.tile_pool(name="xg_p", bufs=3) as xg_p,
            tc.tile_pool(name="ht_p", bufs=2) as ht_p,
            tc.tile_pool(name="sg_p", bufs=2) as sg_p,
            tc.tile_pool(name="y_p", bufs=3) as y_p,
            tc.tile_pool(name="work_p", bufs=2) as work_p,
            tc.tile_pool(name="ps", bufs=4, space="PSUM") as ps_pool,
        ):
            gt_sb = const_p.tile([128, KC, E], BF16)
            bias_sb = const_p.tile([128, E], F32)
            w_sb = const_p.tile([128, ntiles_total], F32)

            def routing(xg_sb, tl, nt, s, ci, jglob):
                """Combine weight of this slot's expert for one token chunk."""
                ps_r = ps_pool.tile([128, nt, E], F32, tag="psr",
                                    bufs=2, name=f"ps_r_{s}_{ci}")
                # partial last tile leaves rows >= ttl unwritten by the
                # matmuls; zero-fill so batched reads are fully defined
                nc.vector.memset(ps_r[:, :nt, :], 0.0)
                for j in range(nt):
                    tt0 = j * 128
                    ttl = min(128, tl - tt0)
                    for k in range(KC):
                        nc.tensor.matmul(
                            ps_r[:ttl, j, :],
                            lhsT=xg_sb[k][:, tt0:tt0 + ttl],
                            rhs=gt_sb[:, k, :],
                            start=(k == 0), stop=(k == KC - 1))
                sc = work_p.tile([128, nt, E], F32, tag="sc",
                                 name=f"sc_{s}_{ci}", padded_shape=[128, 4, E])
                nc.scalar.activation(sc[:, :nt, :], ps_r[:, :nt, :], SIG)
                biased = work_p.tile([128, nt, E], F32, tag="biased",
                                     name=f"biased_{s}_{ci}",
                                     padded_shape=[128, 4, E])
                nc.vector.tensor_tensor(
                    biased[:, :nt, :], sc[:, :nt, :],
                    bias_sb[:, None, :].to_broadcast([128, nt, E]),
                    mybir.AluOpType.add)
                m8 = work_p.tile([128, nt, 8], F32, tag="m8",
                                 name=f"m8_{s}_{ci}", padded_shape=[128, 4, 8])
                sel = work_p.tile([128, nt, E], F32, tag="sel",
                                  name=f"sel_{s}_{ci}",
                                  padded_shape=[128, 4, E])
                for j in range(nt):
                    nc.vector.max(m8[:, j, :], biased[:, j, :])
                for j in range(nt):
                    nc.vector.tensor_scalar(
                        sel[:, j, :], biased[:, j, :],
                        m8[:, j, TOPK - 1:TOPK], None,
                        op0=mybir.AluOpType.is_ge)
                picked = work_p.tile([128, nt, E], F32, tag="picked",
                                     name=f"picked_{s}_{ci}",
                                     padded_shape=[128, 4, E])
                nc.vector.tensor_mul(
                    picked[:, :nt, :], sel[:, :nt, :], sc[:, :nt, :])
                denom = work_p.tile([128, nt], F32, tag="denom",
                                    name=f"denom_{s}_{ci}",
                                    padded_shape=[128, 4])
                nc.vector.reduce_sum(
                    denom[:, :nt], picked[:, :nt, :], axis=mybir.AxisListType.X)
                recip = work_p.tile([128, nt], F32, tag="recip",
                                    name=f"recip_{s}_{ci}",
                                    padded_shape=[128, 4])
                nc.vector.reciprocal(recip[:, :nt], denom[:, :nt])
                # slot expert score is column s (host permutation)
                nc.vector.tensor_mul(
                    w_sb[:, jglob:jglob + nt], sc[:, :nt, s], recip[:, :nt])

            def evac_stage1(ps_g, ps_u, ht_sb, fi, tl):
                """ht[:, fi, :tl] = silu(g) * u = g * sigmoid(g) * u."""
                sgm = sg_p.tile([128, 512], F32, tag="sgm", name=f"sgm_{fi}")
                nc.scalar.activation(sgm[:, :tl], ps_g[:, :tl], SIG)
                gsg = sg_p.tile([128, 512], F32, tag="gsg", name=f"gsg_{fi}")
                nc.vector.tensor_mul(gsg[:, :tl], sgm[:, :tl], ps_g[:, :tl])
                nc.vector.tensor_mul(
                    ht_sb[:, fi, :tl], gsg[:, :tl], ps_u[:, :tl])

            jglob = 0
            for s in range(S):
                cap = caps[s]
                off = sum(caps[:s])
                chunks = chunk_lists[s]

                # k=0 weights split into g/u halves so the first matmul's DMA
                # dependency is small; k>=1 combined to halve the issue count
                w13g0 = w13_p.tile([128, F], BF16, tag="w13g0",
                                   name=f"w13g0_{s}")
                w13u0 = w13_p.tile([128, F], BF16, tag="w13u0",
                                   name=f"w13u0_{s}")
                nc.sync.dma_start(w13g0[:], w13[s, 0:128, 0:F])
                nc.sync.dma_start(w13u0[:], w13[s, 0:128, F:2 * F])
                w13k = [None] + [w13_p.tile([128, 2 * F], BF16,
                                            tag=f"w13_{k}",
                                            name=f"w13_sb_{s}_{k}")
                                 for k in range(1, KC)]
                for k in range(1, KC):
                    nc.sync.dma_start(
                        w13k[k][:], w13[s, k * 128:(k + 1) * 128, :])

                def gv(k, fi):
                    if k == 0:
                        return w13g0[:, fi * 128:(fi + 1) * 128]
                    return w13k[k][:, fi * 128:(fi + 1) * 128]

                def uv(k, fi):
                    if k == 0:
                        return w13u0[:, fi * 128:(fi + 1) * 128]
                    return w13k[k][:, F + fi * 128:F + (fi + 1) * 128]

                tch0 = 0
                for ci, tl in enumerate(chunks):
                    t0 = off + tch0
                    tch0 += tl
                    nt = math.ceil(tl / 128)
                    ramp = (s == 0 and ci == 0)

                    xg_big = xg_p.tile([128, KC, 512], BF16, tag="xg",
                                       name=f"xg_sb_{s}_{ci}")
                    xg_sb = [xg_big[:, k, :] for k in range(KC)]
                    if ramp:
                        # per-k DMAs so the PE can consume k-chunks as they
                        # stream in during the cold start
                        for k in range(KC):
                            nc.scalar.dma_start(
                                xg_big[:, k, :tl], xgt_r[:, k, t0:t0 + tl])
                    else:
                        nc.sync.dma_start(
                            xg_big[:, :, :tl], xgt_r[:, :, t0:t0 + tl])
                    if ramp:
                        # routing consts + slot-0 w2 AFTER the ramp-critical
                        # xg tiles (a big early w2 transfer would stall the
                        # first matmuls behind it on the shared DMA engines)
                        nc.scalar.dma_start(
                            gt_sb[:],
                            gtp.rearrange("(ko p) e -> p ko e", p=128))
                        nc.scalar.dma_start(bias_sb[:], biasp[:])
                    if ci == 0:
                        w2_sb = w2_p.tile([128, FC, H], BF16, tag="w2",
                                          name=f"w2_sb_{s}")
                        nc.scalar.dma_start(
                            w2_sb[:],
                            w2t[s].rearrange("(ko p) h -> p ko h", p=128))

                    ht_sb = ht_p.tile([128, FC, 512], BF16, tag="ht")

                    if not ramp:
                        # routing first: its ACT+DVE chain then completes
                        # during stage 1, well before stage 2 consumes w_sb
                        routing(xg_sb, tl, nt, s, ci, jglob)

                    # ---- stage 1: hT[f,t] = silu(x@w1.T).T * (x@w3.T).T ----
                    if ramp:
                        # k OUTER across all fi: the PE consumes each
                        # weight/activation k-chunk as it streams in.
                        # 8 live PSUM tiles across the three tags.
                        tags = ["ps1", "ps1", "ps1", "psy",
                                "psy", "psy", "psr", "psr"]
                        ps8 = [ps_pool.tile([128, 512], F32, tag=tags[i],
                                            bufs=(2 if tags[i] == "psr"
                                                  else 3),
                                            name=f"ps_ramp_{i}")
                               for i in range(8)]
                        ps_gs = ps8[0::2]
                        ps_us = ps8[1::2]
                        for k in range(KC):
                            for fi in range(FC):
                                nc.tensor.matmul(
                                    ps_gs[fi][:, :tl], lhsT=gv(k, fi),
                                    rhs=xg_sb[k][:, :tl],
                                    start=(k == 0), stop=(k == KC - 1))
                                nc.tensor.matmul(
                                    ps_us[fi][:, :tl], lhsT=uv(k, fi),
                                    rhs=xg_sb[k][:, :tl],
                                    start=(k == 0), stop=(k == KC - 1))
                        for fi in range(FC):
                            evac_stage1(ps_gs[fi], ps_us[fi], ht_sb, fi, tl)
                        # ramp routing last (needs every xg k-chunk anyway)
                        routing(xg_sb, tl, nt, s, ci, jglob)
                    else:
                        # fi sequential, k inner: only 2 PSUM tiles live
                        for fi in range(FC):
                            ps_g = ps_pool.tile([128, 512], F32, tag="ps1",
                                                bufs=3,
                                                name=f"ps_g_{s}_{ci}_{fi}")
                            ps_u = ps_pool.tile([128, 512], F32, tag="ps1",
                                                bufs=3,
                                                name=f"ps_u_{s}_{ci}_{fi}")
                            for k in range(KC):
                                nc.tensor.matmul(
                                    ps_g[:, :tl], lhsT=gv(k, fi),
                                    rhs=xg_sb[k][:, :tl],
                                    start=(k == 0), stop=(k == KC - 1))
                                nc.tensor.matmul(
                                    ps_u[:, :tl], lhsT=uv(k, fi),
                                    rhs=xg_sb[k][:, :tl],
                                    start=(k == 0), stop=(k == KC - 1))
                            evac_stage1(ps_g, ps_u, ht_sb, fi, tl)

                    # ---- stage 2: y[t,h] = w[t] * sum_f hT[f,t]*w2T[f,h] ---
                    for j in range(nt):
                        tt0 = j * 128
                        ttl = min(128, tl - tt0)
                        y_sb = y_p.tile([128, H], BF16, tag="y")
                        for hh in range(2):
                            ps_y = ps_pool.tile([128, 512], F32, tag="psy",
                                                bufs=3,
                                                name=f"ps_y_{s}_{ci}_{j}_{hh}")
                            for kf in range(FC):
                                nc.tensor.matmul(
                                    ps_y[:ttl],
                                    lhsT=ht_sb[:, kf, tt0:tt0 + ttl],
                                    rhs=w2_sb[:, kf, hh * 512:(hh + 1) * 512],
                                    start=(kf == 0), stop=(kf == FC - 1))
                            nc.vector.tensor_scalar(
                                y_sb[:ttl, hh * 512:(hh + 1) * 512],
                                ps_y[:ttl],
                                w_sb[:ttl, jglob + j:jglob + j + 1], None,
                                op0=mybir.AluOpType.mult)
                        nc.sync.dma_start(
                            yg[t0 + tt0:t0 + tt0 + ttl, :], y_sb[:ttl, :])
                    jglob += nt

    nc.compile()
    return nc


def _moe_nc(caps):
    key = ("moe", caps)
    if key not in _nc_cache:
        _nc_cache[key] = _build_moe(caps)
    return _nc_cache[key]


def kernel(hidden_states, gate_w, bias, w1, w3, w2):
    x = np.ascontiguousarray(np.asarray(hidden_states, dtype=np.float32))
    gate_w = np.asarray(gate_w, dtype=np.float32)
    bias = np.asarray(bias, dtype=np.float32)
    w1 = np.asarray(w1, dtype=np.float32)
    w3 = np.asarray(w3, dtype=np.float32)
    w2 = np.asarray(w2, dtype=np.float32)

    # ---- Host dispatch: fp32 routing decides token->expert placement ----
    logits = x @ gate_w.T                                # [T, E]
    scores = 1.0 / (1.0 + np.exp(-logits))
    biased = scores + bias[None, :]
    topi = np.argpartition(-biased, TOPK - 1, axis=1)[:, :TOPK]  # [T, K] sets
    sel = np.zeros((T, E), dtype=bool)
    sel[np.arange(T)[:, None], topi] = True
    idx_per_e = [np.nonzero(sel[:, e])[0] for e in range(E)]
    counts = np.array([len(ix) for ix in idx_per_e])
    caps, placement = _plan_slots(counts)
    S = len(caps)
    offs = [sum(caps[:si]) for si in range(S)]
    global LAST_CAPS
    LAST_CAPS = caps
    CT = sum(caps)

    xT = np.ascontiguousarray(x.T)                       # [H, T]
    xT16 = xT.astype(ml_dtypes.bfloat16)
    gT16 = np.ascontiguousarray(gate_w.T).astype(ml_dtypes.bfloat16)

    in_maps = []
    for c in range(NCORES):
        slot_experts = [p[0] for p in placement[c]]
        idx_pad = np.zeros(CT, dtype=np.int64)
        for si, (e, st, ln) in enumerate(placement[c]):
            if ln:
                idx_pad[offs[si]:offs[si] + ln] = idx_per_e[e][st:st + ln]
        xgt = np.ascontiguousarray(xT16[:, idx_pad])     # [H, CT] bf16
        w13t = np.stack([
            np.ascontiguousarray(
                np.concatenate([w1[e].T, w3[e].T], axis=1))
            for e in slot_experts]).astype(ml_dtypes.bfloat16)  # [S, H, 2F]
        w2t = np.stack(
            [np.ascontiguousarray(w2[e].T) for e in slot_experts]
        ).astype(ml_dtypes.bfloat16)
        perm = slot_experts + [e for e in range(E) if e not in slot_experts]
        gtp = np.ascontiguousarray(gT16[:, perm])        # [H, E] bf16
        biasp = np.ascontiguousarray(
            np.broadcast_to(np.asarray(bias)[perm][None, :],
                            (128, E))).astype(np.float32)
        in_maps.append(
            {"w13t": w13t, "w2t": w2t, "xgt": xgt, "gtp": gtp,
             "biasp": biasp})

    # ---- Single SPMD launch: routing weights + expert FFN ----
    ncB = _moe_nc(caps)
    res = run_bass_kernel_spmd(ncB, in_maps, core_ids=list(range(NCORES)))

    # ---- Host combine: scatter-add ----
    out = np.zeros((T, H), dtype=np.float32)
    for c in range(NCORES):
        for si, (e, st, ln) in enumerate(placement[c]):
            if ln:
                ix = idx_per_e[e][st:st + ln]
                out[ix] += res.results[c]["yg"][offs[si]:offs[si] + ln
                                                ].astype(np.float32)
    return out


# revision 23
# speedup vs baseline: 1.2028x; 1.0006x over previous
"""MiniMax-M2 MoE kernel for 8 Trainium2 NeuronCores.

Single-launch expert-parallel design:
  Host (data movement / dispatch only): fp32 routing decides WHICH tokens go
    to WHICH expert (indices only), gathers tokens per expert into two static
    slots per core (2 experts per core, capacities = exact max expert load),
    pre-transposes/casts weights to bf16.
  Device (all output-value arithmetic, one SPMD launch):
    - per slot, recompute router scores for the slot's gathered tokens
      (logits -> sigmoid -> top-4 threshold on bias-corrected scores ->
      renormalized combine weight of the slot's own expert; the host permutes
      the gate matrix per core so the slot expert is always column 0/1),
    - SwiGLU FFN (bf16 matmuls) and combine-weight scaling.  silu(g) is
      computed as g * sigmoid(g) so the Activation engine only ever needs the
      sigmoid table (one LoadActFuncSet instead of thrashing Silu<->Sigmoid).
  Host: scatter-add per-expert outputs into [T, H] in expert order.
"""

import math

import ml_dtypes
import numpy as np

import concourse.bass as bass
import concourse.tile as tile
from concourse import bacc, mybir
from concourse.bass_utils import run_bass_kernel_spmd

T, H, F, E, TOPK = 4096, 1024, 512, 16, 4
NCORES = 8
KC = H // 128   # contraction chunks (hidden dim)
FC = F // 128   # stage-2 contraction chunks
F32 = mybir.dt.float32
BF16 = mybir.dt.bfloat16

_nc_cache: dict = {}
LAST_CAPS = (832, 492, 512, 354)  # caps used by the most recent kernel() call


def _plan_slots(counts: np.ndarray):
    """Choose per-core slot capacities and expert-piece placement.

    Experts are cut into at most two pieces (primary, remainder).  Slot type
    0 holds heavy-expert primaries (cap A), type 2 light primaries (cap C);
    the remainders are ranked and split between types 1 and 3.  The (A, C)
    cut points are searched to minimize modeled PE time: stage-1 cost scales
    with total capacity, stage-2/routing with ceil(cap/128) tiles.

    Returns (caps, placement) where placement[core] is a list of
    (expert, tok_start, length) per slot (length may be 0).
    """
    E_ = len(counts)
    order = np.argsort(-counts, kind="stable")
    heavy = [int(e) for e in order[:NCORES]]
    light = [int(e) for e in order[NCORES:]]
    c0 = int(counts[heavy[0]])
    c8 = int(counts[light[0]])

    def plan_cost(caps):
        ct = sum(caps)
        tiles = sum(math.ceil(cp / 128) for cp in caps if cp)
        return 64 * ct + (8 * 512 + 8 * E_) * tiles

    def build(A, C):
        pieces_b = []  # (expert, start, len) remainders
        for e in heavy:
            if counts[e] > A:
                pieces_b.append((e, A, int(counts[e]) - A))
        for e in light:
            if counts[e] > C:
                pieces_b.append((e, C, int(counts[e]) - C))
        if len(pieces_b) > 2 * NCORES:
            return None
        pieces_b.sort(key=lambda p: -p[2])
        bs = pieces_b[:NCORES]
        ds = pieces_b[NCORES:]
        a = min(c0, A)
        b = bs[0][2] if bs else 0
        c = min(c8, C)
        d = ds[0][2] if ds else 0
        caps = (a, b, c, d)
        # piece -> core assignment avoiding same expert twice on one core
        placement = [[None] * 4 for _ in range(NCORES)]
        for i in range(NCORES):
            placement[i][0] = (heavy[i], 0, min(int(counts[heavy[i]]), A))
            placement[i][2] = (light[i], 0, min(int(counts[light[i]]), C))
        for sl, plist in ((1, bs), (3, ds)):
            free = set(range(NCORES))
            for e, st, ln in plist:
                cand = [i for i in free
                        if e != placement[i][0][0] and e != placement[i][2][0]
                        and (placement[i][1] is None or
                             placement[i][1][0] != e)]
                if not cand:
                    return None
                i = cand[0]
                free.discard(i)
                placement[i][sl] = (e, st, ln)
        return caps, placement

    best = None
    lo_a = (c0 + 1) // 2
    lo_c = (c8 + 1) // 2
    cands = [(c0, c8)]
    for A in range(lo_a, c0 + 1, 2):
        for C in range(lo_c, c8 + 1, 2):
            cands.append((A, C))
    for A, C in cands:
        got = build(A, C)
        if got is None:
            continue
        caps, placement = got
        cost = plan_cost(caps)
        if best is None or cost < best[0]:
            best = (cost, caps, placement)
    _, caps, placement = best
    # drop zero-cap slots; fill empty kept slots with a zero-length piece of
    # some expert not already used by that core (perm needs distinct experts)
    keep = [si for si in range(4) if caps[si] > 0]
    caps_k = tuple(caps[si] for si in keep)
    placement_k = []
    for pl in placement:
        row = []
        used = {p[0] for p in pl if p is not None}
        for si in keep:
            p = pl[si]
            if p is None:
                e_fill = next(e for e in range(E_) if e not in used)
                used.add(e_fill)
                p = (e_fill, 0, 0)
            row.append(p)
        placement_k.append(row)
    return caps_k, placement_k


def _chunk_sizes(cap: int, rem_first: bool) -> list[int]:
    """Split cap into <=512-sized chunks; remainder first or last."""
    n_full, rem = divmod(cap, 512)
    sizes = [512] * n_full
    if rem:
        if rem_first:
            sizes = [rem] + sizes
        else:
            sizes = sizes + [rem]
    return sizes


def _build_moe(caps: tuple[int, ...]):
    """One-launch MoE FFN + on-device combine weights.

    Inputs per core (S = len(caps) expert slots):
      w13t  [S, H, 2F] bf16  per-slot hstack(w1[e].T, w3[e].T)
      w2t   [S, F, H]  bf16  per-slot w2[e].T
      xgt   [H, CT]    bf16  gathered tokens (transposed), CT = sum(caps)
      gtp   [H, E]     bf16  gate_w.T, columns permuted so that column s is
                             slot s's expert
      biasp [128, E]   f32   e_score_correction_bias, same permutation,
                             broadcast to 128 partitions
    Output:
      yg    [CT, H]    bf16  combine-weighted expert outputs per gathered token
    """
    S = len(caps)
    CT = sum(caps)
    chunk_lists = [_chunk_sizes(cap, rem_first=False) for cap in caps]
    ntiles_total = sum(math.ceil(tl / 128)
                       for chunks in chunk_lists for tl in chunks)

    nc = bacc.Bacc("TRN2", target_bir_lowering=False, debug=False,
                   num_devices=NCORES)
    w13 = nc.dram_tensor("w13t", [S, H, 2 * F], BF16,
                         kind="ExternalInput").ap()
    w2t = nc.dram_tensor("w2t", [S, F, H], BF16, kind="ExternalInput").ap()
    xgt = nc.dram_tensor("xgt", [H, CT], BF16, kind="ExternalInput").ap()
    gtp = nc.dram_tensor("gtp", [H, E], BF16, kind="ExternalInput").ap()
    biasp = nc.dram_tensor("biasp", [128, E], F32, kind="ExternalInput").ap()
    yg = nc.dram_tensor("yg", [CT, H], BF16, kind="ExternalOutput").ap()

    xgt_r = xgt.rearrange("(ko p) t -> p ko t", p=128)
    SIG = mybir.ActivationFunctionType.Sigmoid

    with tile.TileContext(nc) as tc:
        with (
            tc.tile_pool(name="const_p", bufs=1) as const_p,
            tc.tile_pool(name="w13_p", bufs=2) as w13_p,
            tc.tile_pool(name="w2_p", bufs=2) as w2_p,
            tc.tile_pool(name="xg_p", bufs=3) as xg_p,
            tc.tile_pool(name="ht_p", bufs=2) as ht_p,
            tc.tile_pool(name="sg_p", bufs=2) as sg_p,
            tc.tile_pool(name="y_p", bufs=3) as y_p,
            tc.tile_pool(name="work_p", bufs=2) as work_p,
            tc.tile_pool(name="ps", bufs=4, space="PSUM") as ps_pool,
        ):
            gt_sb = const_p.tile([128, KC, E], BF16)
            bias_sb = const_p.tile([128, E], F32)
            w_sb = const_p.tile([128, ntiles_total], F32)

            def routing(xg_sb, tl, nt, s, ci, jglob):
                """Combine weight of this slot's expert for one token chunk."""
                ps_r = ps_pool.tile([128, nt, E], F32, tag="psr",
                                    bufs=2, name=f"ps_r_{s}_{ci}")
                # partial last tile leaves rows >= ttl unwritten by the
                # matmuls; zero-fill so batched reads are fully defined
                nc.vector.memset(ps_r[:, :nt, :], 0.0)
                for j in range(nt):
                    tt0 = j * 128
                    ttl = min(128, tl - tt0)
                    for k in range(KC):
                        nc.tensor.matmul(
                            ps_r[:ttl, j, :],
                            lhsT=xg_sb[k][:, tt0:tt0 + ttl],
                            rhs=gt_sb[:, k, :],
                            start=(k == 0), stop=(k == KC - 1))
                sc = work_p.tile([128, nt, E], F32, tag="sc",
                                 name=f"sc_{s}_{ci}", padded_shape=[128, 4, E])
                nc.scalar.activation(sc[:, :nt, :], ps_r[:, :nt, :], SIG)
                biased = work_p.tile([128, nt, E], F32, tag="biased",
                                     name=f"biased_{s}_{ci}",
                                     padded_shape=[128, 4, E])
                nc.vector.tensor_tensor(
                    biased[:, :nt, :], sc[:, :nt, :],
                    bias_sb[:, None, :].to_broadcast([128, nt, E]),
                    mybir.AluOpType.add)
                m8 = work_p.tile([128, nt, 8], F32, tag="m8",
                                 name=f"m8_{s}_{ci}", padded_shape=[128, 4, 8])
                sel = work_p.tile([128, nt, E], F32, tag="sel",
                                  name=f"sel_{s}_{ci}",
                                  padded_shape=[128, 4, E])
                for j in range(nt):
                    nc.vector.max(m8[:, j, :], biased[:, j, :])
                for j in range(nt):
                    nc.vector.tensor_scalar(
                        sel[:, j, :], biased[:, j, :],
                        m8[:, j, TOPK - 1:TOPK], None,
                        op0=mybir.AluOpType.is_ge)
                picked = work_p.tile([128, nt, E], F32, tag="picked",
                                     name=f"picked_{s}_{ci}",
                                     padded_shape=[128, 4, E])
                nc.vector.tensor_mul(
                    picked[:, :nt, :], sel[:, :nt, :], sc[:, :nt, :])
                denom = work_p.tile([128, nt], F32, tag="denom",
                                    name=f"denom_{s}_{ci}",
                                    padded_shape=[128, 4])
                nc.vector.reduce_sum(
                    denom[:, :nt], picked[:, :nt, :], axis=mybir.AxisListType.X)
                recip = work_p.tile([128, nt], F32, tag="recip",
                                    name=f"recip_{s}_{ci}",
                                    padded_shape=[128, 4])
                nc.vector.reciprocal(recip[:, :nt], denom[:, :nt])
                # slot expert score is column s (host permutation)
                nc.vector.tensor_mul(
                    w_sb[:, jglob:jglob + nt], sc[:, :nt, s], recip[:, :nt])

            def evac_stage1(ps_g, ps_u, ht_sb, fi, tl):
                """ht[:, fi, :tl] = silu(g) * u = g * sigmoid(g) * u."""
                sgm = sg_p.tile([128, 512], F32, tag="sgm", name=f"sgm_{fi}")
                nc.scalar.activation(sgm[:, :tl], ps_g[:, :tl], SIG)
                gsg = sg_p.tile([128, 512], F32, tag="gsg", name=f"gsg_{fi}")
                nc.vector.tensor_mul(gsg[:, :tl], sgm[:, :tl], ps_g[:, :tl])
                nc.vector.tensor_mul(
                    ht_sb[:, fi, :tl], gsg[:, :tl], ps_u[:, :tl])

            jglob = 0
            for s in range(S):
                cap = caps[s]
                off = sum(caps[:s])
                chunks = chunk_lists[s]

                # k=0 weights split into g/u halves so the first matmul's DMA
                # dependency is small; k>=1 combined to halve the issue count
                w13g0 = w13_p.tile([128, F], BF16, tag="w13g0",
                                   name=f"w13g0_{s}")
                w13u0 = w13_p.tile([128, F], BF16, tag="w13u0",
                                   name=f"w13u0_{s}")
                nc.sync.dma_start(w13g0[:], w13[s, 0:128, 0:F])
                nc.sync.dma_start(w13u0[:], w13[s, 0:128, F:2 * F])
                w13k = [None] + [w13_p.tile([128, 2 * F], BF16,
                                            tag=f"w13_{k}",
                                            name=f"w13_sb_{s}_{k}")
                                 for k in range(1, KC)]
                for k in range(1, KC):
                    nc.sync.dma_start(
                        w13k[k][:], w13[s, k * 128:(k + 1) * 128, :])

                def gv(k, fi):
                    if k == 0:
                        return w13g0[:, fi * 128:(fi + 1) * 128]
                    return w13k[k][:, fi * 128:(fi + 1) * 128]

                def uv(k, fi):
                    if k == 0:
                        return w13u0[:, fi * 128:(fi + 1) * 128]
                    return w13k[k][:, F + fi * 128:F + (fi + 1) * 128]

                tch0 = 0
                for ci, tl in enumerate(chunks):
                    t0 = off + tch0
                    tch0 += tl
                    nt = math.ceil(tl / 128)
                    ramp = (s == 0 and ci == 0)

                    xg_big = xg_p.tile([128, KC, 512], BF16, tag="xg",
                                       name=f"xg_sb_{s}_{ci}")
                    xg_sb = [xg_big[:, k, :] for k in range(KC)]
                    if ramp:
                        # per-k DMAs so the PE can consume k-chunks as they
                        # stream in during the cold start
                        for k in range(KC):
                            nc.scalar.dma_start(
                                xg_big[:, k, :tl], xgt_r[:, k, t0:t0 + tl])
                    else:
                        nc.sync.dma_start(
                            xg_big[:, :, :tl], xgt_r[:, :, t0:t0 + tl])
                    if ramp:
                        # routing consts + slot-0 w2 AFTER the ramp-critical
                        # xg tiles (a big early w2 transfer would stall the
                        # first matmuls behind it on the shared DMA engines)
                        nc.scalar.dma_start(
                            gt_sb[:],
                            gtp.rearrange("(ko p) e -> p ko e", p=128))
                        nc.scalar.dma_start(bias_sb[:], biasp[:])
                    if ci == 0:
                        w2_sb = w2_p.tile([128, FC, H], BF16, tag="w2",
                                          name=f"w2_sb_{s}")
                        nc.scalar.dma_start(
                            w2_sb[:],
                            w2t[s].rearrange("(ko p) h -> p ko h", p=128))

                    ht_sb = ht_p.tile([128, FC, 512], BF16, tag="ht")

                    if not ramp:
                        # routing first: its ACT+DVE chain then completes
                        # during stage 1, well before stage 2 consumes w_sb
                        routing(xg_sb, tl, nt, s, ci, jglob)

                    # ---- stage 1: hT[f,t] = silu(x@w1.T).T * (x@w3.T).T ----
                    if ramp:
                        # k OUTER across all fi: the PE consumes each
                        # weight/activation k-chunk as it streams in.
                        # 8 live PSUM tiles across the three tags.
                        tags = ["ps1", "ps1", "ps1", "psy",
                                "psy", "psy", "psr", "psr"]
                        ps8 = [ps_pool.tile([128, 512], F32, tag=tags[i],
                                            bufs=(2 if tags[i] == "psr"
                                                  else 3),
                                            name=f"ps_ramp_{i}")
                               for i in range(8)]
                        ps_gs = ps8[0::2]
                        ps_us = ps8[1::2]
                        for k in range(KC):
                            for fi in range(FC):
                                nc.tensor.matmul(
                                    ps_gs[fi][:, :tl], lhsT=gv(k, fi),
                                    rhs=xg_sb[k][:, :tl],
                                    start=(k == 0), stop=(k == KC - 1))
                                nc.tensor.matmul(
                                    ps_us[fi][:, :tl], lhsT=uv(k, fi),
                                    rhs=xg_sb[k][:, :tl],
                                    start=(k == 0), stop=(k == KC - 1))
                        for fi in range(FC):
                            evac_stage1(ps_gs[fi], ps_us[fi], ht_sb, fi, tl)
                        # ramp routing last (needs every xg k-chunk anyway)
                        routing(xg_sb, tl, nt, s, ci, jglob)
                    else:
                        # fi sequential, k inner: only 2 PSUM tiles live
                        for fi in range(FC):
                            ps_g = ps_pool.tile([128, 512], F32, tag="ps1",
                                                bufs=3,
                                                name=f"ps_g_{s}_{ci}_{fi}")
                            ps_u = ps_pool.tile([128, 512], F32, tag="ps1",
                                                bufs=3,
                                                name=f"ps_u_{s}_{ci}_{fi}")
                            for k in range(KC):
                                nc.tensor.matmul(
                                    ps_g[:, :tl], lhsT=gv(k, fi),
                                    rhs=xg_sb[k][:, :tl],
                                    start=(k == 0), stop=(k == KC - 1))
                                nc.tensor.matmul(
                                    ps_u[:, :tl], lhsT=uv(k, fi),
                                    rhs=xg_sb[k][:, :tl],
                                    start=(k == 0), stop=(k == KC - 1))
                            evac_stage1(ps_g, ps_u, ht_sb, fi, tl)

                    # ---- stage 2: y[t,h] = w[t] * sum_f hT[f,t]*w2T[f,h] ---
                    last_chunk = (s == S - 1 and ci == len(chunks) - 1)
                    for j in range(nt):
                        tt0 = j * 128
                        ttl = min(128, tl - tt0)
                        wj = w_sb[:ttl, jglob + j:jglob + j + 1]
                        y_sb = y_p.tile([128, H], BF16, tag="y")
                        ps_ys = []
                        for hh in range(2):
                            ps_y = ps_pool.tile([128, 512], F32, tag="psy",
                                                bufs=3,
                                                name=f"ps_y_{s}_{ci}_{j}_{hh}")
                            ps_ys.append(ps_y)
                            for kf in range(FC):
                                nc.tensor.matmul(
                                    ps_y[:ttl],
                                    lhsT=ht_sb[:, kf, tt0:tt0 + ttl],
                                    rhs=w2_sb[:, kf, hh * 512:(hh + 1) * 512],
                                    start=(kf == 0), stop=(kf == FC - 1))
                            if hh == 0 or not (last_chunk and j == nt - 1):
                                nc.vector.tensor_scalar(
                                    y_sb[:ttl, hh * 512:(hh + 1) * 512],
                                    ps_y[:ttl], wj, None,
                                    op0=mybir.AluOpType.mult)
                        if last_chunk and j == nt - 1:
                            # final tile: drain the second half as two quarter
                            # pieces on parallel engines/queues to shorten the
                            # end-of-kernel DMA latency chain
                            rows = slice(t0 + tt0, t0 + tt0 + ttl)
                            nc.sync.dma_start(
                                yg[rows, 0:512], y_sb[:ttl, 0:512])
                            nc.scalar.activation(
                                y_sb[:ttl, 512:768], ps_ys[1][:ttl, 0:256],
                                mybir.ActivationFunctionType.Copy,
                                scale=wj)
                            nc.scalar.dma_start(
                                yg[rows, 512:768], y_sb[:ttl, 512:768])
                            ybq = y_p.tile([128, 256], BF16, tag="ybq",
                                           bufs=1, name="ybq_last")
                            nc.vector.tensor_scalar(
                                ybq[:ttl, :], ps_ys[1][:ttl, 256:512],
                                wj, None, op0=mybir.AluOpType.mult)
                            nc.sync.dma_start(
                                yg[rows, 768:1024], ybq[:ttl, :])
                        else:
                            nc.sync.dma_start(
                                yg[t0 + tt0:t0 + tt0 + ttl, :], y_sb[:ttl, :])
                    jglob += nt

    nc.compile()
    return nc


def _moe_nc(caps):
    key = ("moe", caps)
    if key not in _nc_cache:
        _nc_cache[key] = _build_moe(caps)
    return _nc_cache[key]


def kernel(hidden_states, gate_w, bias, w1, w3, w2):
    x = np.ascontiguousarray(np.asarray(hidden_states, dtype=np.float32))
    gate_w = np.asarray(gate_w, dtype=np.float32)
    bias = np.asarray(bias, dtype=np.float32)
    w1 = np.asarray(w1, dtype=np.float32)
    w3 = np.asarray(w3, dtype=np.float32)
    w2 = np.asarray(w2, dtype=np.float32)

    # ---- Host dispatch: fp32 routing decides token->expert placement ----
    logits = x @ gate_w.T                                # [T, E]
    scores = 1.0 / (1.0 + np.exp(-logits))
    biased = scores + bias[None, :]
    topi = np.argpartition(-biased, TOPK - 1, axis=1)[:, :TOPK]  # [T, K] sets
    sel = np.zeros((T, E), dtype=bool)
    sel[np.arange(T)[:, None], topi] = True
    idx_per_e = [np.nonzero(sel[:, e])[0] for e in range(E)]
    counts = np.array([len(ix) for ix in idx_per_e])
    caps, placement = _plan_slots(counts)
    S = len(caps)
    offs = [sum(caps[:si]) for si in range(S)]
    global LAST_CAPS
    LAST_CAPS = caps
    CT = sum(caps)

    xT = np.ascontiguousarray(x.T)                       # [H, T]
    xT16 = xT.astype(ml_dtypes.bfloat16)
    gT16 = np.ascontiguousarray(gate_w.T).astype(ml_dtypes.bfloat16)

    in_maps = []
    for c in range(NCORES):
        slot_experts = [p[0] for p in placement[c]]
        idx_pad = np.zeros(CT, dtype=np.int64)
        for si, (e, st, ln) in enumerate(placement[c]):
            if ln:
                idx_pad[offs[si]:offs[si] + ln] = idx_per_e[e][st:st + ln]
        xgt = np.ascontiguousarray(xT16[:, idx_pad])     # [H, CT] bf16
        w13t = np.stack([
            np.ascontiguousarray(
                np.concatenate([w1[e].T, w3[e].T], axis=1))
            for e in slot_experts]).astype(ml_dtypes.bfloat16)  # [S, H, 2F]
        w2t = np.stack(
            [np.ascontiguousarray(w2[e].T) for e in slot_experts]
        ).astype(ml_dtypes.bfloat16)
        perm = slot_experts + [e for e in range(E) if e not in slot_experts]
        gtp = np.ascontiguousarray(gT16[:, perm])        # [H, E] bf16
        biasp = np.ascontiguousarray(
            np.broadcast_to(np.asarray(bias)[perm][None, :],
                            (128, E))).astype(np.float32)
        in_maps.append(
            {"w13t": w13t, "w2t": w2t, "xgt": xgt, "gtp": gtp,
             "biasp": biasp})

    # ---- Single SPMD launch: routing weights + expert FFN ----
    ncB = _moe_nc(caps)
    res = run_bass_kernel_spmd(ncB, in_maps, core_ids=list(range(NCORES)))

    # ---- Host combine: scatter-add ----
    out = np.zeros((T, H), dtype=np.float32)
    for c in range(NCORES):
        for si, (e, st, ln) in enumerate(placement[c]):
            if ln:
                ix = idx_per_e[e][st:st + ln]
                out[ix] += res.results[c]["yg"][offs[si]:offs[si] + ln
                                                ].astype(np.float32)
    return out
